# revision 34
# baseline (speedup 1.0000x reference)
"""Trainium2 Bass kernel for nn_Attention_64235530879146 (v9).

Per core (B=1, C=512, T=1024, 8 heads of ch=64, 32 groups):
    xn = GroupNorm(x) * gn_weight + gn_bias          # [C, T]
    qkv = W1 @ xn + b1                               # [3C, T]
    per head: St[s,t] = (k*sc)^T (q*sc),  sc = ch**-0.25
              Wt = exp(St);  a = (V Wt) / r,  r[t] = sum_s Wt[s,t]
    out = a + x

Sharding: pure data-parallel over batch (8 elements on 8 cores, no
collectives).

Structure (measured on HW: ~166us vs ~204us for the v1 baseline):
  - Head-pair pipeline: pair j streams scores -> exp -> AV (ACT-paced)
    while q/k for pair j+1 trickle through a 1-bank psum staging slot
    and pair 0 overlaps the V-chunk production.
  - PSUM (8 banks): scores 2x[128,1024] fp32 (4), av 3x[65,512] (3),
    qkv/v staging + GN + last-pair-4th-accumulator (1).
  - exp: ACT Exp on [128,1024] psum tiles; (st,n) units in
    DVE_EXP_UNITS instead run a Schraudolph exp on the DVE -
    tensor_scalar computes round(S*128*log2(e) + (127*128 - 5.7)) into
    an int16-bitcast view of the bf16 wt tile, i.e. 2^y assembled in
    the bf16 exponent field. The +-3% mantissa ripple cancels almost
    entirely through the softmax normalization (r uses the same
    approximate weights); measured end-to-end rel err ~5.5e-4.
  - AV keeps the ones-column trick (lhsT = [v^T | 1], M=65) so r rides
    out of the AV matmul in psum row 64 for free.
  - Epilogue per (h, n): one copy evacuates av+r to SBUF bf16 (frees
    the av psum slot immediately - this, not engine time, was the
    pipeline stall); r reshaped [1,512]->[16,32] by DMA for a
    lane-parallel DVE reciprocal; 1/r broadcast to 64 partitions by a
    K=1 PE matmul (ones[1,64]^T @ rinv_row) into psum - no slow DMA
    broadcast; out = (av*rinv + b1v) + x via tensor_mul +
    scalar_tensor_tensor. v-bias folds in exactly via
    sum_s b1v*Wt[s,t] = b1v*r[t].
  - GroupNorm rstd via bit-trick + 2 Newton steps on DVE (no second
    ACT table set); the exp table is preloaded by a dummy activation
    during the DMA phase; 12 warmup matmuls un-throttle the PE HAM.
  - Loads: x and w1t-q on sync, x2/x3 and w1t-k on the scalar HWDGE
    queue, w1t-v on gpsimd; x_hd (residual copy) after.

Pitfalls baked into this design (cost a lot of debugging): custom-DVE
ops (reciprocal_approx_fast) and ISA ops (partition_broadcast) have
access patterns that are INVISIBLE to the Tile dependency tracker, and
post-hoc add_dep_helper edges do not take - they race and corrupt
results. TensorHandle.bitcast views ARE tracked. Matmul psum output
must be fp32. Only sync/scalar (HWDGE) and gpsimd (SWDGE, slow) can
issue DMAs; SWDGE transfers run ~3-6x slower.

Matmul inputs are bf16 (fp32 PSUM accumulate); w1 transposed and cast
on the host (pure layout prep).
"""
import numpy as np

GROUPS = 32
HEADS = 8
EPS = 1e-5
C = 512
T = 1024
CH = C // HEADS            # 64
SCALE = float(CH) ** -0.25
N_CORES = 8

DVE_EXP_UNITS = {(2, 0), (6, 1)}   # (st, n) units whose exp runs on DVE
EPI_V2 = True              # bisect: v2-style epilogue (DMA r-chain)
LAST_INLINE = True        # bisect: v2-style deferred last pair
N_WARM = 12                # PE warmup matmuls (HAM un-throttle)

LOG2E_128 = 184.6650558    # log2(e) * 128  (bf16 exponent scale)
SCHRAUD_B = 16250.3        # 127*128 - 5.7 (centres the 2^frac error band)


def _build_nc():
    import concourse.bass as bass
    import concourse.mybir as mybir
    import concourse.tile as tile
    from concourse import bacc
    from concourse.tile_rust import add_dep_helper

    f32 = mybir.dt.float32
    bf16 = mybir.dt.bfloat16
    i16 = mybir.dt.int16
    i32 = mybir.dt.int32
    Alu = mybir.AluOpType
    Act = mybir.ActivationFunctionType

    nc = bacc.Bacc("TRN2", target_bir_lowering=False, debug=False)

    x_d = nc.declare_dram_parameter("x", [C, T], f32, isOutput=False)
    w1t_d = nc.declare_dram_parameter("w1t", [C, 3 * C], bf16, isOutput=False)
    b1r_d = nc.declare_dram_parameter("b1r", [128, 8], f32, isOutput=False)
    b1vh_d = nc.declare_dram_parameter("b1vh", [64, 8], f32, isOutput=False)
    gnw_d = nc.declare_dram_parameter("gnw", [128, 4], f32, isOutput=False)
    gnb_d = nc.declare_dram_parameter("gnb", [128, 4], f32, isOutput=False)
    ind16_d = nc.declare_dram_parameter("ind16", [128, 8], f32, isOutput=False)
    indT_d = nc.declare_dram_parameter("indT", [8, 128], f32, isOutput=False)
    out_d = nc.declare_dram_parameter("out", [C, T], f32, isOutput=True)

    with tile.TileContext(nc) as tc:
        with (
            tc.tile_pool(name="cst", bufs=1) as cst,
            tc.tile_pool(name="work", bufs=2) as work,
            tc.tile_pool(name="wtp", bufs=4) as wtp,
            tc.tile_pool(name="outp", bufs=4) as outp,
            tc.tile_pool(name="ps", bufs=1, space="PSUM") as ps,
        ):
            # ---------------- PE warmup (no data deps) ----------------
            wuA = cst.tile([128, 128], bf16)
            nc.vector.memset(wuA, 0.0)
            wuB = cst.tile([128, 512], bf16)
            nc.vector.memset(wuB, 0.0)
            wu_ps = ps.tile([64, 512], f32, tag="av", bufs=3, name="warm")
            for i in range(N_WARM):
                nc.tensor.matmul(
                    out=wu_ps[0:64, :], lhsT=wuA[:, 0:64], rhs=wuB,
                    start=True, stop=True
                )

            # ---------------- loads ----------------
            # tiny consts first on the gpsimd queue
            b1r_sb = cst.tile([128, 8], f32)
            nc.gpsimd.dma_start(out=b1r_sb, in_=b1r_d[:, :])
            b1vh_sb = cst.tile([64, 8], f32)
            nc.gpsimd.dma_start(out=b1vh_sb, in_=b1vh_d[:, :])
            gnw_sb = cst.tile([128, 4], f32)
            nc.gpsimd.dma_start(out=gnw_sb, in_=gnw_d[:, :])
            gnb_sb = cst.tile([128, 4], f32)
            nc.gpsimd.dma_start(out=gnb_sb, in_=gnb_d[:, :])
            ind16 = cst.tile([128, 8], f32)
            nc.gpsimd.dma_start(out=ind16, in_=ind16_d[:, :])
            indT = cst.tile([8, 128], f32)
            nc.gpsimd.dma_start(out=indT, in_=indT_d[:, :])

            # x on sync+gpsimd; w1t q/k on the scalar queue (boot critical path)
            xv = x_d.ap().rearrange("(i p) t -> i p t", p=128)
            x_sb = cst.tile([128, 4, T], f32)
            w1t_sb = cst.tile([128, 4, 3 * C], bf16)
            w1tv = w1t_d.ap().rearrange("(i p) o -> p i o", p=128)
            nc.sync.dma_start(out=x_sb[:, 0, :], in_=xv[0])
            nc.sync.dma_start(out=x_sb[:, 1, :], in_=xv[1])
            nc.scalar.dma_start(out=x_sb[:, 2, :], in_=xv[2])
            nc.scalar.dma_start(out=x_sb[:, 3, :], in_=xv[3])
            nc.sync.dma_start(out=w1t_sb[:, :, 0:C], in_=w1tv[:, :, 0:C])
            nc.scalar.dma_start(out=w1t_sb[:, :, C : 2 * C], in_=w1tv[:, :, C : 2 * C])
            nc.gpsimd.dma_start(out=w1t_sb[:, :, 2 * C :], in_=w1tv[:, :, 2 * C :])
            # head-aligned residual copy of x (needed ~25us in)
            x_hd = cst.tile([64, 8, T], f32)
            nc.sync.dma_start(out=x_hd, in_=x_d.ap().rearrange("(h p) t -> p h t", p=64))

            # residual with v-bias prefolded: x_hd2[:,h,:] = x_hd[:,h,:] + b1v[h]
            x_hd2 = cst.tile([64, 8, T], f32)
            for hh in range(8):
                nc.vector.tensor_scalar(
                    out=x_hd2[:, hh, :], in0=x_hd[:, hh, :],
                    scalar1=b1vh_sb[:, hh : hh + 1], scalar2=None, op0=Alu.add,
                )

            # preload the exp table while DMAs stream
            dumm = cst.tile([8, 2], f32)
            nc.vector.memset(dumm, 0.0)
            nc.scalar.activation(
                out=dumm[:, 1:2], in_=dumm[:, 0:1], func=Act.Exp, bias=0.0, scale=1.0
            )

            # ---------------- GroupNorm ----------------
            rhs3 = cst.tile([128, 4, 3], f32)
            for i in range(4):
                st6 = work.tile([128, 2, 6], f32, tag="st6")
                nc.vector.bn_stats(out=st6[:, 0, :], in_=x_sb[:, i, 0:512])
                nc.vector.bn_stats(out=st6[:, 1, :], in_=x_sb[:, i, 512:1024])
                mv = work.tile([128, 2], f32, tag="mv")
                nc.vector.bn_aggr(out=mv, in_=st6)
                nc.vector.tensor_copy(out=rhs3[:, i, 0:2], in_=mv)
                nc.vector.tensor_mul(rhs3[:, i, 2:3], mv[:, 0:1], mv[:, 0:1])

            stats_ps = ps.tile([8, 12], f32, tag="qkv", name="stats_ps")
            for i in range(4):
                nc.tensor.matmul(
                    out=stats_ps[:, 3 * i : 3 * i + 3],
                    lhsT=ind16,
                    rhs=rhs3[:, i, :],
                    start=True,
                    stop=True,
                )
            sg = cst.tile([8, 12], f32)
            nc.vector.tensor_copy(out=sg, in_=stats_ps)
            musig = cst.tile([8, 2, 4], f32)
            mu_v = sg.rearrange("p (i three) -> p i three", three=3)
            nc.vector.tensor_copy(out=musig[:, 0, :], in_=mu_v[:, :, 0])
            var_g = cst.tile([8, 4], f32)
            nc.vector.tensor_add(var_g, mu_v[:, :, 1], mu_v[:, :, 2])
            mu2 = cst.tile([8, 4], f32)
            nc.vector.tensor_mul(mu2, mu_v[:, :, 0], mu_v[:, :, 0])
            nc.vector.tensor_sub(var_g, var_g, mu2)
            # rstd = 1/sqrt(var+eps): bit-trick seed + 2 Newton steps (DVE)
            ve = cst.tile([8, 4], f32)
            nc.vector.tensor_scalar(
                out=ve, in0=var_g, scalar1=EPS, scalar2=None, op0=Alu.add
            )
            t_i = cst.tile([8, 4], i32)
            nc.vector.tensor_scalar(
                out=t_i, in0=ve.bitcast(i32), scalar1=1, scalar2=None,
                op0=Alu.logical_shift_right,
            )
            t_x = cst.tile([8, 4], i32)
            nc.vector.tensor_scalar(
                out=t_x, in0=t_i, scalar1=-1, scalar2=None, op0=Alu.bitwise_xor
            )
            y_i = cst.tile([8, 4], i32)
            nc.vector.tensor_scalar(
                out=y_i, in0=t_x, scalar1=0x5F3759E0, scalar2=None, op0=Alu.add
            )
            y0 = y_i.bitcast(f32)
            t2 = cst.tile([8, 4], f32)
            nc.vector.tensor_mul(t2, y0, y0)
            nc.vector.tensor_mul(t2, t2, ve)
            nc.vector.tensor_scalar(
                out=t2, in0=t2, scalar1=-0.5, scalar2=1.5, op0=Alu.mult, op1=Alu.add
            )
            y1 = cst.tile([8, 4], f32)
            nc.vector.tensor_mul(y1, y0, t2)
            t3 = cst.tile([8, 4], f32)
            nc.vector.tensor_mul(t3, y1, y1)
            nc.vector.tensor_mul(t3, t3, ve)
            nc.vector.tensor_scalar(
                out=t3, in0=t3, scalar1=-0.5, scalar2=1.5, op0=Alu.mult, op1=Alu.add
            )
            nc.vector.tensor_mul(musig[:, 1, :], y1, t3)

            # broadcast (mu, rstd) to channels; fold gn affine; xn bf16
            xn_sb = cst.tile([128, 4, T], bf16)
            af = cst.tile([128, 4, 2], f32)
            for i in range(4):
                musig_ps = ps.tile([128, 2], f32, tag="av", bufs=3, name=f"musig_ps{i}")
                nc.tensor.matmul(
                    out=musig_ps, lhsT=indT, rhs=musig[:, :, i], start=True, stop=True
                )
                nc.vector.tensor_mul(af[:, i, 0:1], gnw_sb[:, i : i + 1], musig_ps[:, 1:2])
                tmp = work.tile([128, 1], f32, tag="tmp1")
                nc.vector.tensor_mul(tmp, musig_ps[:, 0:1], af[:, i, 0:1])
                nc.vector.tensor_sub(af[:, i, 1:2], gnb_sb[:, i : i + 1], tmp)
                nc.vector.tensor_scalar(
                    out=xn_sb[:, i, :],
                    in0=x_sb[:, i, :],
                    scalar1=af[:, i, 0:1],
                    scalar2=af[:, i, 1:2],
                    op0=Alu.mult,
                    op1=Alu.add,
                )

            # ---------------- QKV machinery ----------------
            q_sb = cst.tile([128, 4, T], bf16)
            k_sb = cst.tile([128, 4, T], bf16)
            vt_sb = cst.tile([128, 8, 8, 65], bf16)
            nc.vector.memset(vt_sb[:, :, :, 64:65], 1.0)

            def emit_qk_boot(j):
                # prologue q_j/k_j through the sc slots, full width
                for oc, dst in ((j, q_sb), (4 + j, k_sb)):
                    qp = ps.tile([128, T], f32, tag="sc", bufs=2, name=f"boot_{oc}")
                    for n in range(2):
                        for i in range(4):
                            nc.tensor.matmul(
                                out=qp[:, 512 * n : 512 * n + 512],
                                lhsT=w1t_sb[:, i, 128 * oc : 128 * oc + 128],
                                rhs=xn_sb[:, i, 512 * n : 512 * n + 512],
                                start=(i == 0),
                                stop=(i == 3),
                            )
                    nc.vector.tensor_scalar(
                        out=dst[:, j, :],
                        in0=qp,
                        scalar1=SCALE,
                        scalar2=b1r_sb[:, oc : oc + 1],
                        op0=Alu.mult,
                        op1=Alu.add,
                    )

            def emit_qk(j):
                # steady-state q_j/k_j in [128,512] halves through the staging slot
                for oc, dst in ((j, q_sb), (4 + j, k_sb)):
                    for n in range(2):
                        qp = ps.tile([128, 512], f32, tag="qkv", name=f"qk_{oc}_{n}")
                        for i in range(4):
                            nc.tensor.matmul(
                                out=qp,
                                lhsT=w1t_sb[:, i, 128 * oc : 128 * oc + 128],
                                rhs=xn_sb[:, i, 512 * n : 512 * n + 512],
                                start=(i == 0),
                                stop=(i == 3),
                            )
                        nc.vector.tensor_scalar(
                            out=dst[:, j, 512 * n : 512 * n + 512],
                            in0=qp,
                            scalar1=SCALE,
                            scalar2=b1r_sb[:, oc : oc + 1],
                            op0=Alu.mult,
                            op1=Alu.add,
                        )

            def emit_v(st):
                vp = ps.tile([128, 512], f32, tag="av", bufs=3, name=f"v_{st}")
                for i in range(4):
                    nc.tensor.matmul(
                        out=vp,
                        lhsT=xn_sb[:, i, 128 * st : 128 * st + 128],
                        rhs=w1t_sb[:, i, 2 * C : 3 * C],
                        start=(i == 0),
                        stop=(i == 3),
                    )
                nc.vector.tensor_copy(
                    out=vt_sb[:, st, :, 0:64],
                    in_=vp.rearrange("p (h c) -> p h c", c=64),
                )

            wt_exp_inst = {}

            ones64 = cst.tile([1, 64], f32)
            nc.vector.memset(ones64, 1.0)

            def epilogue(j, h, n, avt, mm_last):
                # evacuate av+r at once (frees the av psum slot immediately);
                # 1/r via [16,32] reshape; broadcast 1/r to 64 partitions with
                # a K=1 PE matmul into psum (no slow DMA broadcast).
                o65 = outp.tile([65, 512], bf16, tag="o65", bufs=4, name=f"o65_{h}_{n}")
                nc.vector.tensor_copy(out=o65, in_=avt[0:65, :])
                rsp = wtp.tile([16, 32], bf16, tag="rsp", bufs=4, name=f"rp_{h}_{n}")
                nc.sync.dma_start(out=rsp, in_=o65[64:65, :])
                rsp2 = wtp.tile([16, 32], f32, tag="rsp2", bufs=4, name=f"rq_{h}_{n}")
                nc.vector.reciprocal(out=rsp2, in_=rsp)
                rrow2 = wtp.tile([1, 512], f32, tag="rrow2", bufs=4, name=f"r2_{h}_{n}")
                nc.sync.dma_start(out=rrow2, in_=rsp2)
                rbc_ps = ps.tile([64, 512], f32, tag="av", bufs=3, name=f"rb_{h}_{n}")
                nc.tensor.matmul(
                    out=rbc_ps, lhsT=ones64, rhs=rrow2, start=True, stop=True
                )
                o_f = outp.tile([64, 512], f32, tag="obf", name=f"ob_{h}_{n}")
                nc.vector.tensor_mul(o_f, o65[0:64, :], rbc_ps)
                out_f = outp.tile([64, 512], f32, tag="of", name=f"of_{h}_{n}")
                nc.gpsimd.tensor_add(
                    out_f, o_f, x_hd2[:, h, 512 * n : 512 * n + 512]
                )
                nc.sync.dma_start(
                    out=out_d[64 * h : 64 * h + 64, 512 * n : 512 * n + 512], in_=out_f
                )

            # ---------------- pipeline ----------------
            emit_qk_boot(0)

            for j in range(HEADS // 2):
                hA, hB = 2 * j, 2 * j + 1
                last = (j == HEADS // 2 - 1) and LAST_INLINE
                av = {
                    (hA, 0): ps.tile([65, 512], f32, tag="av", bufs=3, name=f"av_{hA}_0"),
                    (hB, 0): ps.tile([65, 512], f32, tag="av", bufs=3, name=f"av_{hB}_0"),
                }
                av_last = {}
                if last:
                    # borrow the idle staging slot for a 4th inline accumulator
                    av[(hA, 1)] = ps.tile([128, 512], f32, tag="qkv", name=f"av_{hA}_1")
                    av[(hB, 1)] = ps.tile([65, 512], f32, tag="av", bufs=3, name=f"av_{hB}_1")
                wts = []

                def emit_av(st, n_range):
                    for n in n_range:
                        for hi, h in enumerate((hA, hB)):
                            mm = nc.tensor.matmul(
                                out=av[(h, n)][0:65, :],
                                lhsT=vt_sb[:, st, h, 0:65],
                                rhs=wts[st][n][:, 512 * hi : 512 * hi + 512],
                                start=(st == 0),
                                stop=(st == 7),
                            )
                            if st == 7:
                                av_last[(h, n)] = mm

                for st in range(8):
                    if j == 0:
                        emit_v(st)
                    wt_pair = []
                    for n in range(2):
                        scn = ps.tile([128, T], f32, tag="sc", bufs=2, name=f"sc_{j}_{st}_{n}")
                        for hi, h in enumerate((hA, hB)):
                            hp = 64 * hi
                            nc.tensor.matmul(
                                out=scn[:, 512 * hi : 512 * hi + 512],
                                lhsT=k_sb[hp : hp + 64, j, 128 * st : 128 * st + 128],
                                rhs=q_sb[hp : hp + 64, j, 512 * n : 512 * n + 512],
                                start=True,
                                stop=True,
                                tile_position=(hp, 0),
                            )
                        wtn = wtp.tile(
                            [128, T], bf16, tag="wt", bufs=20, name=f"wt_{j}_{st}_{n}"
                        )
                        if (st, n) in DVE_EXP_UNITS:
                            # write through a bitcast handle: untracked by Tile,
                            # so AV reads get explicit deps (wt_exp_inst)
                            ei = nc.vector.tensor_scalar(
                                out=wtn.bitcast(i16),
                                in0=scn,
                                scalar1=LOG2E_128,
                                scalar2=SCHRAUD_B,
                                op0=Alu.mult,
                                op1=Alu.add,
                            )
                            wt_exp_inst[id(wtn)] = ei
                        else:
                            nc.scalar.activation(
                                out=wtn, in_=scn, func=Act.Exp, bias=0.0, scale=1.0
                            )
                        wt_pair.append(wtn)
                    wts.append(wt_pair)

                    # AV deferred by one st so the PE FIFO never waits on the
                    # exp of the st it just scored (head-of-line blocking)
                    n_range = (0, 1) if last else (0,)
                    if st > 0:
                        emit_av(st - 1, n_range)
                    # q/k for the next pair mid-pair, while the staging slot
                    # and the engines have slack
                    if st == 3 and j < HEADS // 2 - 1:
                        emit_qk(j + 1)
                if True:
                    emit_av(7, n_range)

                if not last:
                    epilogue(j, hA, 0, av[(hA, 0)], av_last[(hA, 0)])
                    av[(hA, 1)] = ps.tile([65, 512], f32, tag="av", bufs=3, name=f"av_{hA}_1")
                    epilogue(j, hB, 0, av[(hB, 0)], av_last[(hB, 0)])
                    av[(hB, 1)] = ps.tile([65, 512], f32, tag="av", bufs=3, name=f"av_{hB}_1")
                    for st in range(8):
                        for hi, h in enumerate((hA, hB)):
                            mm = nc.tensor.matmul(
                                out=av[(h, 1)][0:65, :],
                                lhsT=vt_sb[:, st, h, 0:65],
                                rhs=wts[st][1][:, 512 * hi : 512 * hi + 512],
                                start=(st == 0),
                                stop=(st == 7),
                            )
                            if st == 7:
                                av_last[(h, 1)] = mm
                    epilogue(j, hA, 1, av[(hA, 1)], av_last[(hA, 1)])
                    epilogue(j, hB, 1, av[(hB, 1)], av_last[(hB, 1)])
                else:
                    for h in (hA, hB):
                        for n in (0, 1):
                            epilogue(j, h, n, av[(h, n)], av_last[(h, n)])

    nc.finalize()
    return nc


def _make_in_maps(inputs):
    x = np.ascontiguousarray(np.asarray(inputs["x"], dtype=np.float32))
    gnw = np.asarray(inputs["gn_weight"], dtype=np.float32)
    gnb = np.asarray(inputs["gn_bias"], dtype=np.float32)
    w1 = np.asarray(inputs["w1"], dtype=np.float32)
    b1 = np.asarray(inputs["b1"], dtype=np.float32)

    import ml_dtypes

    B = x.shape[0]
    w1t = np.ascontiguousarray(w1[:, :, 0].T).astype(ml_dtypes.bfloat16)  # [C, 3C]
    b1r = np.ascontiguousarray(b1[: 2 * C].reshape(8, 128).T) * (float(CH) ** -0.25)  # [128, 8], pre-scaled
    b1vh = np.ascontiguousarray(b1[2 * C :].reshape(8, 64).T)       # [64, 8]
    gnw_r = np.ascontiguousarray(gnw.reshape(4, 128).T)             # [128, 4]
    gnb_r = np.ascontiguousarray(gnb.reshape(4, 128).T)             # [128, 4]

    ind16 = np.zeros((128, 8), np.float32)
    indT = np.zeros((8, 128), np.float32)
    for g in range(8):
        ind16[16 * g : 16 * g + 16, g] = 1.0 / 16.0
        indT[g, 16 * g : 16 * g + 16] = 1.0

    in_maps = []
    for b in range(B):
        in_maps.append(
            {
                "x": np.ascontiguousarray(x[b].reshape(C, T)),
                "w1t": w1t,
                "b1r": b1r,
                "b1vh": b1vh,
                "gnw": gnw_r,
                "gnb": gnb_r,
                "ind16": ind16,
                "indT": indT,
            }
        )
    return in_maps


def _gather(results, x_shape):
    B, Cc, H, W = x_shape
    out = np.empty((B, Cc, H, W), dtype=np.float32)
    for b in range(B):
        out[b] = results[b]["out"].reshape(Cc, H, W)
    return out


def kernel(**inputs):
    from concourse.bass_utils import run_bass_kernel_spmd

    nc = _build_nc()
    in_maps = _make_in_maps(inputs)
    res = run_bass_kernel_spmd(nc, in_maps, core_ids=list(range(N_CORES)))
    return _gather(res.results, np.asarray(inputs["x"]).shape)


# revision 35
# speedup vs baseline: 1.0159x; 1.0159x over previous
"""Trainium2 Bass kernel for nn_Attention_64235530879146 (v9).

Per core (B=1, C=512, T=1024, 8 heads of ch=64, 32 groups):
    xn = GroupNorm(x) * gn_weight + gn_bias          # [C, T]
    qkv = W1 @ xn + b1                               # [3C, T]
    per head: St[s,t] = (k*sc)^T (q*sc),  sc = ch**-0.25
              Wt = exp(St);  a = (V Wt) / r,  r[t] = sum_s Wt[s,t]
    out = a + x

Sharding: pure data-parallel over batch (8 elements on 8 cores, no
collectives).

Structure (measured on HW: ~166us vs ~204us for the v1 baseline):
  - Head-pair pipeline: pair j streams scores -> exp -> AV (ACT-paced)
    while q/k for pair j+1 trickle through a 1-bank psum staging slot
    and pair 0 overlaps the V-chunk production.
  - PSUM (8 banks): scores 2x[128,1024] fp32 (4), av 3x[65,512] (3),
    qkv/v staging + GN + last-pair-4th-accumulator (1).
  - exp: ACT Exp on [128,1024] psum tiles; (st,n) units in
    DVE_EXP_UNITS instead run a Schraudolph exp on the DVE -
    tensor_scalar computes round(S*128*log2(e) + (127*128 - 5.7)) into
    an int16-bitcast view of the bf16 wt tile, i.e. 2^y assembled in
    the bf16 exponent field. The +-3% mantissa ripple cancels almost
    entirely through the softmax normalization (r uses the same
    approximate weights); measured end-to-end rel err ~5.5e-4.
  - AV keeps the ones-column trick (lhsT = [v^T | 1], M=65) so r rides
    out of the AV matmul in psum row 64 for free.
  - Epilogue per (h, n): one copy evacuates av+r to SBUF bf16 (frees
    the av psum slot immediately - this, not engine time, was the
    pipeline stall); r reshaped [1,512]->[16,32] by DMA for a
    lane-parallel DVE reciprocal; 1/r broadcast to 64 partitions by a
    K=1 PE matmul (ones[1,64]^T @ rinv_row) into psum - no slow DMA
    broadcast; out = (av*rinv + b1v) + x via tensor_mul +
    scalar_tensor_tensor. v-bias folds in exactly via
    sum_s b1v*Wt[s,t] = b1v*r[t].
  - GroupNorm rstd via bit-trick + 2 Newton steps on DVE (no second
    ACT table set); the exp table is preloaded by a dummy activation
    during the DMA phase; 12 warmup matmuls un-throttle the PE HAM.
  - Loads: x and w1t-q on sync, x2/x3 and w1t-k on the scalar HWDGE
    queue, w1t-v on gpsimd; x_hd (residual copy) after.

Pitfalls baked into this design (cost a lot of debugging): custom-DVE
ops (reciprocal_approx_fast) and ISA ops (partition_broadcast) have
access patterns that are INVISIBLE to the Tile dependency tracker, and
post-hoc add_dep_helper edges do not take - they race and corrupt
results. TensorHandle.bitcast views ARE tracked. Matmul psum output
must be fp32. Only sync/scalar (HWDGE) and gpsimd (SWDGE, slow) can
issue DMAs; SWDGE transfers run ~3-6x slower.

Matmul inputs are bf16 (fp32 PSUM accumulate); w1 transposed and cast
on the host (pure layout prep).
"""
import numpy as np

GROUPS = 32
HEADS = 8
EPS = 1e-5
C = 512
T = 1024
CH = C // HEADS            # 64
SCALE = float(CH) ** -0.25
N_CORES = 8

DVE_EXP_UNITS = {(2, 0), (6, 1)}   # (st, n) units whose exp runs on DVE
EPI_V2 = True              # bisect: v2-style epilogue (DMA r-chain)
LAST_INLINE = True        # bisect: v2-style deferred last pair
N_WARM = 12                # PE warmup matmuls (HAM un-throttle)

LOG2E_128 = 184.6650558    # log2(e) * 128  (bf16 exponent scale)
SCHRAUD_B = 16250.3        # 127*128 - 5.7 (centres the 2^frac error band)


def _build_nc():
    import concourse.bass as bass
    import concourse.mybir as mybir
    import concourse.tile as tile
    from concourse import bacc
    from concourse.tile_rust import add_dep_helper

    f32 = mybir.dt.float32
    bf16 = mybir.dt.bfloat16
    i16 = mybir.dt.int16
    i32 = mybir.dt.int32
    Alu = mybir.AluOpType
    Act = mybir.ActivationFunctionType

    nc = bacc.Bacc("TRN2", target_bir_lowering=False, debug=False)

    x_d = nc.declare_dram_parameter("x", [C, T], f32, isOutput=False)
    w1t_d = nc.declare_dram_parameter("w1t", [C, 3 * C], bf16, isOutput=False)
    b1r_d = nc.declare_dram_parameter("b1r", [128, 8], f32, isOutput=False)
    b1vh_d = nc.declare_dram_parameter("b1vh", [64, 8], f32, isOutput=False)
    gnw_d = nc.declare_dram_parameter("gnw", [128, 4], f32, isOutput=False)
    gnb_d = nc.declare_dram_parameter("gnb", [128, 4], f32, isOutput=False)
    ind16_d = nc.declare_dram_parameter("ind16", [128, 8], f32, isOutput=False)
    indT_d = nc.declare_dram_parameter("indT", [8, 128], f32, isOutput=False)
    out_d = nc.declare_dram_parameter("out", [C, T], f32, isOutput=True)

    with tile.TileContext(nc) as tc:
        with (
            tc.tile_pool(name="cst", bufs=1) as cst,
            tc.tile_pool(name="work", bufs=2) as work,
            tc.tile_pool(name="wtp", bufs=4) as wtp,
            tc.tile_pool(name="outp", bufs=4) as outp,
            tc.tile_pool(name="ps", bufs=1, space="PSUM") as ps,
        ):
            # ---------------- PE warmup (no data deps) ----------------
            wuA = cst.tile([128, 128], bf16)
            nc.vector.memset(wuA, 0.0)
            wuB = cst.tile([128, 512], bf16)
            nc.vector.memset(wuB, 0.0)
            wu_ps = ps.tile([64, 512], f32, tag="av", bufs=3, name="warm")
            for i in range(N_WARM):
                nc.tensor.matmul(
                    out=wu_ps[0:64, :], lhsT=wuA[:, 0:64], rhs=wuB,
                    start=True, stop=True
                )

            # ---------------- loads ----------------
            # tiny consts first on the gpsimd queue
            b1r_sb = cst.tile([128, 8], f32)
            nc.gpsimd.dma_start(out=b1r_sb, in_=b1r_d[:, :])
            b1vh_sb = cst.tile([64, 8], f32)
            nc.gpsimd.dma_start(out=b1vh_sb, in_=b1vh_d[:, :])
            gnw_sb = cst.tile([128, 4], f32)
            nc.gpsimd.dma_start(out=gnw_sb, in_=gnw_d[:, :])
            gnb_sb = cst.tile([128, 4], f32)
            nc.gpsimd.dma_start(out=gnb_sb, in_=gnb_d[:, :])
            ind16 = cst.tile([128, 8], f32)
            nc.gpsimd.dma_start(out=ind16, in_=ind16_d[:, :])
            indT = cst.tile([8, 128], f32)
            nc.gpsimd.dma_start(out=indT, in_=indT_d[:, :])

            # x on sync+gpsimd; w1t q/k on the scalar queue (boot critical path)
            xv = x_d.ap().rearrange("(i p) t -> i p t", p=128)
            x_sb = cst.tile([128, 4, T], f32)
            w1t_sb = cst.tile([128, 4, 3 * C], bf16)
            w1tv = w1t_d.ap().rearrange("(i p) o -> p i o", p=128)
            nc.sync.dma_start(out=x_sb[:, 0, :], in_=xv[0])
            nc.sync.dma_start(out=x_sb[:, 1, :], in_=xv[1])
            nc.scalar.dma_start(out=x_sb[:, 2, :], in_=xv[2])
            nc.scalar.dma_start(out=x_sb[:, 3, :], in_=xv[3])
            nc.sync.dma_start(out=w1t_sb[:, :, 0:C], in_=w1tv[:, :, 0:C])
            nc.scalar.dma_start(out=w1t_sb[:, :, C : 2 * C], in_=w1tv[:, :, C : 2 * C])
            nc.gpsimd.dma_start(out=w1t_sb[:, :, 2 * C :], in_=w1tv[:, :, 2 * C :])
            # head-aligned residual copy of x (needed ~25us in)
            x_hd = cst.tile([64, 8, T], f32)
            nc.sync.dma_start(out=x_hd, in_=x_d.ap().rearrange("(h p) t -> p h t", p=64))

            # residual with v-bias prefolded: x_hd2[:,h,:] = x_hd[:,h,:] + b1v[h]
            x_hd2 = cst.tile([64, 8, T], f32)
            for hh in range(8):
                nc.vector.tensor_scalar(
                    out=x_hd2[:, hh, :], in0=x_hd[:, hh, :],
                    scalar1=b1vh_sb[:, hh : hh + 1], scalar2=None, op0=Alu.add,
                )

            # preload the exp table while DMAs stream
            dumm = cst.tile([8, 2], f32)
            nc.vector.memset(dumm, 0.0)
            nc.scalar.activation(
                out=dumm[:, 1:2], in_=dumm[:, 0:1], func=Act.Exp, bias=0.0, scale=1.0
            )

            # ---------------- GroupNorm ----------------
            rhs3 = cst.tile([128, 4, 3], f32)
            for i in range(4):
                st6 = work.tile([128, 2, 6], f32, tag="st6")
                nc.vector.bn_stats(out=st6[:, 0, :], in_=x_sb[:, i, 0:512])
                nc.vector.bn_stats(out=st6[:, 1, :], in_=x_sb[:, i, 512:1024])
                mv = work.tile([128, 2], f32, tag="mv")
                nc.vector.bn_aggr(out=mv, in_=st6)
                nc.vector.tensor_copy(out=rhs3[:, i, 0:2], in_=mv)
                nc.vector.tensor_mul(rhs3[:, i, 2:3], mv[:, 0:1], mv[:, 0:1])

            stats_ps = ps.tile([8, 12], f32, tag="qkv", name="stats_ps")
            for i in range(4):
                nc.tensor.matmul(
                    out=stats_ps[:, 3 * i : 3 * i + 3],
                    lhsT=ind16,
                    rhs=rhs3[:, i, :],
                    start=True,
                    stop=True,
                )
            sg = cst.tile([8, 12], f32)
            nc.vector.tensor_copy(out=sg, in_=stats_ps)
            musig = cst.tile([8, 2, 4], f32)
            mu_v = sg.rearrange("p (i three) -> p i three", three=3)
            nc.vector.tensor_copy(out=musig[:, 0, :], in_=mu_v[:, :, 0])
            var_g = cst.tile([8, 4], f32)
            nc.vector.tensor_add(var_g, mu_v[:, :, 1], mu_v[:, :, 2])
            mu2 = cst.tile([8, 4], f32)
            nc.vector.tensor_mul(mu2, mu_v[:, :, 0], mu_v[:, :, 0])
            nc.vector.tensor_sub(var_g, var_g, mu2)
            # rstd = 1/sqrt(var+eps): bit-trick seed + 2 Newton steps (DVE)
            ve = cst.tile([8, 4], f32)
            nc.vector.tensor_scalar(
                out=ve, in0=var_g, scalar1=EPS, scalar2=None, op0=Alu.add
            )
            t_i = cst.tile([8, 4], i32)
            nc.vector.tensor_scalar(
                out=t_i, in0=ve.bitcast(i32), scalar1=1, scalar2=None,
                op0=Alu.logical_shift_right,
            )
            t_x = cst.tile([8, 4], i32)
            nc.vector.tensor_scalar(
                out=t_x, in0=t_i, scalar1=-1, scalar2=None, op0=Alu.bitwise_xor
            )
            y_i = cst.tile([8, 4], i32)
            nc.vector.tensor_scalar(
                out=y_i, in0=t_x, scalar1=0x5F3759E0, scalar2=None, op0=Alu.add
            )
            y0 = y_i.bitcast(f32)
            t2 = cst.tile([8, 4], f32)
            nc.vector.tensor_mul(t2, y0, y0)
            nc.vector.tensor_mul(t2, t2, ve)
            nc.vector.tensor_scalar(
                out=t2, in0=t2, scalar1=-0.5, scalar2=1.5, op0=Alu.mult, op1=Alu.add
            )
            y1 = cst.tile([8, 4], f32)
            nc.vector.tensor_mul(y1, y0, t2)
            t3 = cst.tile([8, 4], f32)
            nc.vector.tensor_mul(t3, y1, y1)
            nc.vector.tensor_mul(t3, t3, ve)
            nc.vector.tensor_scalar(
                out=t3, in0=t3, scalar1=-0.5, scalar2=1.5, op0=Alu.mult, op1=Alu.add
            )
            nc.vector.tensor_mul(musig[:, 1, :], y1, t3)

            # broadcast (mu, rstd) to channels; fold gn affine; xn bf16
            xn_sb = cst.tile([128, 4, T], bf16)
            af = cst.tile([128, 4, 2], f32)
            for i in range(4):
                musig_ps = ps.tile([128, 2], f32, tag="av", bufs=3, name=f"musig_ps{i}")
                nc.tensor.matmul(
                    out=musig_ps, lhsT=indT, rhs=musig[:, :, i], start=True, stop=True
                )
                nc.vector.tensor_mul(af[:, i, 0:1], gnw_sb[:, i : i + 1], musig_ps[:, 1:2])
                tmp = work.tile([128, 1], f32, tag="tmp1")
                nc.vector.tensor_mul(tmp, musig_ps[:, 0:1], af[:, i, 0:1])
                nc.vector.tensor_sub(af[:, i, 1:2], gnb_sb[:, i : i + 1], tmp)
                nc.vector.tensor_scalar(
                    out=xn_sb[:, i, :],
                    in0=x_sb[:, i, :],
                    scalar1=af[:, i, 0:1],
                    scalar2=af[:, i, 1:2],
                    op0=Alu.mult,
                    op1=Alu.add,
                )

            # ---------------- QKV machinery ----------------
            q_sb = cst.tile([128, 4, T], bf16)
            k_sb = cst.tile([128, 4, T], bf16)
            vt_sb = cst.tile([128, 8, 8, 65], bf16)
            nc.vector.memset(vt_sb[:, :, :, 64:65], 1.0)

            def emit_qk_boot(j):
                # prologue q_j/k_j through the sc slots, full width
                for oc, dst in ((j, q_sb), (4 + j, k_sb)):
                    qp = ps.tile([128, T], f32, tag="sc", bufs=2, name=f"boot_{oc}")
                    for n in range(2):
                        for i in range(4):
                            nc.tensor.matmul(
                                out=qp[:, 512 * n : 512 * n + 512],
                                lhsT=w1t_sb[:, i, 128 * oc : 128 * oc + 128],
                                rhs=xn_sb[:, i, 512 * n : 512 * n + 512],
                                start=(i == 0),
                                stop=(i == 3),
                            )
                    nc.vector.tensor_scalar(
                        out=dst[:, j, :],
                        in0=qp,
                        scalar1=SCALE,
                        scalar2=b1r_sb[:, oc : oc + 1],
                        op0=Alu.mult,
                        op1=Alu.add,
                    )

            def emit_qk(j):
                # steady-state q_j/k_j in [128,512] halves through the staging slot
                for oc, dst in ((j, q_sb), (4 + j, k_sb)):
                    for n in range(2):
                        qp = ps.tile([128, 512], f32, tag="qkv", name=f"qk_{oc}_{n}")
                        for i in range(4):
                            nc.tensor.matmul(
                                out=qp,
                                lhsT=w1t_sb[:, i, 128 * oc : 128 * oc + 128],
                                rhs=xn_sb[:, i, 512 * n : 512 * n + 512],
                                start=(i == 0),
                                stop=(i == 3),
                            )
                        nc.vector.tensor_scalar(
                            out=dst[:, j, 512 * n : 512 * n + 512],
                            in0=qp,
                            scalar1=SCALE,
                            scalar2=b1r_sb[:, oc : oc + 1],
                            op0=Alu.mult,
                            op1=Alu.add,
                        )

            def emit_v(st):
                vp = ps.tile([128, 512], f32, tag="av", bufs=3, name=f"v_{st}")
                for i in range(4):
                    nc.tensor.matmul(
                        out=vp,
                        lhsT=xn_sb[:, i, 128 * st : 128 * st + 128],
                        rhs=w1t_sb[:, i, 2 * C : 3 * C],
                        start=(i == 0),
                        stop=(i == 3),
                    )
                nc.vector.tensor_copy(
                    out=vt_sb[:, st, :, 0:64],
                    in_=vp.rearrange("p (h c) -> p h c", c=64),
                )

            wt_exp_inst = {}

            ones64 = cst.tile([1, 64], f32)
            nc.vector.memset(ones64, 1.0)

            def epilogue(j, h, n, avt, mm_last):
                # evacuate av+r at once (frees the av psum slot immediately);
                # 1/r via [16,32] reshape; broadcast 1/r to 64 partitions with
                # a K=1 PE matmul into psum (no slow DMA broadcast).
                o65 = outp.tile([65, 512], bf16, tag="o65", bufs=4, name=f"o65_{h}_{n}")
                nc.vector.tensor_copy(out=o65, in_=avt[0:65, :])
                rsp = wtp.tile([16, 32], bf16, tag="rsp", bufs=4, name=f"rp_{h}_{n}")
                nc.sync.dma_start(out=rsp, in_=o65[64:65, :])
                rsp2 = wtp.tile([16, 32], f32, tag="rsp2", bufs=4, name=f"rq_{h}_{n}")
                nc.vector.reciprocal(out=rsp2, in_=rsp)
                rrow2 = wtp.tile([1, 512], f32, tag="rrow2", bufs=4, name=f"r2_{h}_{n}")
                nc.sync.dma_start(out=rrow2, in_=rsp2)
                rbc_ps = ps.tile([64, 512], f32, tag="av", bufs=3, name=f"rb_{h}_{n}")
                nc.tensor.matmul(
                    out=rbc_ps, lhsT=ones64, rhs=rrow2, start=True, stop=True
                )
                o_f = outp.tile([64, 512], f32, tag="obf", name=f"ob_{h}_{n}")
                nc.vector.tensor_mul(o_f, o65[0:64, :], rbc_ps)
                out_f = outp.tile([64, 512], f32, tag="of", name=f"of_{h}_{n}")
                nc.gpsimd.tensor_add(
                    out_f, o_f, x_hd2[:, h, 512 * n : 512 * n + 512]
                )
                nc.sync.dma_start(
                    out=out_d[64 * h : 64 * h + 64, 512 * n : 512 * n + 512], in_=out_f
                )

            # ---------------- pipeline ----------------
            emit_qk_boot(0)

            for j in range(HEADS // 2):
                hA, hB = 2 * j, 2 * j + 1
                last = (j == HEADS // 2 - 1) and LAST_INLINE
                av = {
                    (hA, 0): ps.tile([65, 512], f32, tag="av", bufs=3, name=f"av_{hA}_0"),
                    (hB, 0): ps.tile([65, 512], f32, tag="av", bufs=3, name=f"av_{hB}_0"),
                }
                av_last = {}
                if last:
                    # borrow the idle staging slot for a 4th inline accumulator
                    av[(hA, 1)] = ps.tile([128, 512], f32, tag="qkv", name=f"av_{hA}_1")
                    av[(hB, 1)] = ps.tile([65, 512], f32, tag="av", bufs=3, name=f"av_{hB}_1")
                wts = []

                def emit_av(st, n_range):
                    for n in n_range:
                        for hi, h in enumerate((hA, hB)):
                            mm = nc.tensor.matmul(
                                out=av[(h, n)][0:65, :],
                                lhsT=vt_sb[:, st, h, 0:65],
                                rhs=wts[st][n][:, 512 * hi : 512 * hi + 512],
                                start=(st == 0),
                                stop=(st == 7),
                            )
                            if st == 7:
                                av_last[(h, n)] = mm

                for st in range(8):
                    if j == 0:
                        emit_v(st)
                    wt_pair = []
                    for n in range(2):
                        scn = ps.tile([128, T], f32, tag="sc", bufs=2, name=f"sc_{j}_{st}_{n}")
                        for hi, h in enumerate((hA, hB)):
                            hp = 64 * hi
                            nc.tensor.matmul(
                                out=scn[:, 512 * hi : 512 * hi + 512],
                                lhsT=k_sb[hp : hp + 64, j, 128 * st : 128 * st + 128],
                                rhs=q_sb[hp : hp + 64, j, 512 * n : 512 * n + 512],
                                start=True,
                                stop=True,
                                tile_position=(hp, 0),
                            )
                        wtn = wtp.tile(
                            [128, T], bf16, tag="wt", bufs=20, name=f"wt_{j}_{st}_{n}"
                        )
                        if (st, n) in DVE_EXP_UNITS:
                            # write through a bitcast handle: untracked by Tile,
                            # so AV reads get explicit deps (wt_exp_inst)
                            ei = nc.vector.tensor_scalar(
                                out=wtn.bitcast(i16),
                                in0=scn,
                                scalar1=LOG2E_128,
                                scalar2=SCHRAUD_B,
                                op0=Alu.mult,
                                op1=Alu.add,
                            )
                            wt_exp_inst[id(wtn)] = ei
                        else:
                            nc.scalar.activation(
                                out=wtn, in_=scn, func=Act.Exp, bias=0.0, scale=1.0
                            )
                        wt_pair.append(wtn)
                    wts.append(wt_pair)

                    # AV deferred by one st so the PE FIFO never waits on the
                    # exp of the st it just scored (head-of-line blocking)
                    n_range = (0, 1) if last else (0,)
                    if st > 0:
                        emit_av(st - 1, n_range)
                if True:
                    emit_av(7, n_range)

                if not last:
                    epilogue(j, hA, 0, av[(hA, 0)], av_last[(hA, 0)])
                    av[(hA, 1)] = ps.tile([65, 512], f32, tag="av", bufs=3, name=f"av_{hA}_1")
                    epilogue(j, hB, 0, av[(hB, 0)], av_last[(hB, 0)])
                    av[(hB, 1)] = ps.tile([65, 512], f32, tag="av", bufs=3, name=f"av_{hB}_1")
                    if j < HEADS // 2 - 1:
                        emit_qk(j + 1)
                    for st in range(8):
                        for hi, h in enumerate((hA, hB)):
                            mm = nc.tensor.matmul(
                                out=av[(h, 1)][0:65, :],
                                lhsT=vt_sb[:, st, h, 0:65],
                                rhs=wts[st][1][:, 512 * hi : 512 * hi + 512],
                                start=(st == 0),
                                stop=(st == 7),
                            )
                            if st == 7:
                                av_last[(h, 1)] = mm
                    epilogue(j, hA, 1, av[(hA, 1)], av_last[(hA, 1)])
                    epilogue(j, hB, 1, av[(hB, 1)], av_last[(hB, 1)])
                else:
                    for h in (hA, hB):
                        for n in (0, 1):
                            epilogue(j, h, n, av[(h, n)], av_last[(h, n)])

    nc.finalize()
    return nc


def _make_in_maps(inputs):
    x = np.ascontiguousarray(np.asarray(inputs["x"], dtype=np.float32))
    gnw = np.asarray(inputs["gn_weight"], dtype=np.float32)
    gnb = np.asarray(inputs["gn_bias"], dtype=np.float32)
    w1 = np.asarray(inputs["w1"], dtype=np.float32)
    b1 = np.asarray(inputs["b1"], dtype=np.float32)

    import ml_dtypes

    B = x.shape[0]
    w1t = np.ascontiguousarray(w1[:, :, 0].T).astype(ml_dtypes.bfloat16)  # [C, 3C]
    b1r = np.ascontiguousarray(b1[: 2 * C].reshape(8, 128).T) * (float(CH) ** -0.25)  # [128, 8], pre-scaled
    b1vh = np.ascontiguousarray(b1[2 * C :].reshape(8, 64).T)       # [64, 8]
    gnw_r = np.ascontiguousarray(gnw.reshape(4, 128).T)             # [128, 4]
    gnb_r = np.ascontiguousarray(gnb.reshape(4, 128).T)             # [128, 4]

    ind16 = np.zeros((128, 8), np.float32)
    indT = np.zeros((8, 128), np.float32)
    for g in range(8):
        ind16[16 * g : 16 * g + 16, g] = 1.0 / 16.0
        indT[g, 16 * g : 16 * g + 16] = 1.0

    in_maps = []
    for b in range(B):
        in_maps.append(
            {
                "x": np.ascontiguousarray(x[b].reshape(C, T)),
                "w1t": w1t,
                "b1r": b1r,
                "b1vh": b1vh,
                "gnw": gnw_r,
                "gnb": gnb_r,
                "ind16": ind16,
                "indT": indT,
            }
        )
    return in_maps


def _gather(results, x_shape):
    B, Cc, H, W = x_shape
    out = np.empty((B, Cc, H, W), dtype=np.float32)
    for b in range(B):
        out[b] = results[b]["out"].reshape(Cc, H, W)
    return out


def kernel(**inputs):
    from concourse.bass_utils import run_bass_kernel_spmd

    nc = _build_nc()
    in_maps = _make_in_maps(inputs)
    res = run_bass_kernel_spmd(nc, in_maps, core_ids=list(range(N_CORES)))
    return _gather(res.results, np.asarray(inputs["x"]).shape)


# revision 36
# speedup vs baseline: 1.0381x; 1.0219x over previous
"""Trainium2 Bass kernel for nn_Attention_64235530879146 (v9).

Per core (B=1, C=512, T=1024, 8 heads of ch=64, 32 groups):
    xn = GroupNorm(x) * gn_weight + gn_bias          # [C, T]
    qkv = W1 @ xn + b1                               # [3C, T]
    per head: St[s,t] = (k*sc)^T (q*sc),  sc = ch**-0.25
              Wt = exp(St);  a = (V Wt) / r,  r[t] = sum_s Wt[s,t]
    out = a + x

Sharding: pure data-parallel over batch (8 elements on 8 cores, no
collectives).

Structure (measured on HW: ~166us vs ~204us for the v1 baseline):
  - Head-pair pipeline: pair j streams scores -> exp -> AV (ACT-paced)
    while q/k for pair j+1 trickle through a 1-bank psum staging slot
    and pair 0 overlaps the V-chunk production.
  - PSUM (8 banks): scores 2x[128,1024] fp32 (4), av 3x[65,512] (3),
    qkv/v staging + GN + last-pair-4th-accumulator (1).
  - exp: ACT Exp on [128,1024] psum tiles; (st,n) units in
    DVE_EXP_UNITS instead run a Schraudolph exp on the DVE -
    tensor_scalar computes round(S*128*log2(e) + (127*128 - 5.7)) into
    an int16-bitcast view of the bf16 wt tile, i.e. 2^y assembled in
    the bf16 exponent field. The +-3% mantissa ripple cancels almost
    entirely through the softmax normalization (r uses the same
    approximate weights); measured end-to-end rel err ~5.5e-4.
  - AV keeps the ones-column trick (lhsT = [v^T | 1], M=65) so r rides
    out of the AV matmul in psum row 64 for free.
  - Epilogue per (h, n): one copy evacuates av+r to SBUF bf16 (frees
    the av psum slot immediately - this, not engine time, was the
    pipeline stall); r reshaped [1,512]->[16,32] by DMA for a
    lane-parallel DVE reciprocal; 1/r broadcast to 64 partitions by a
    K=1 PE matmul (ones[1,64]^T @ rinv_row) into psum - no slow DMA
    broadcast; out = (av*rinv + b1v) + x via tensor_mul +
    scalar_tensor_tensor. v-bias folds in exactly via
    sum_s b1v*Wt[s,t] = b1v*r[t].
  - GroupNorm rstd via bit-trick + 2 Newton steps on DVE (no second
    ACT table set); the exp table is preloaded by a dummy activation
    during the DMA phase; 12 warmup matmuls un-throttle the PE HAM.
  - Loads: x and w1t-q on sync, x2/x3 and w1t-k on the scalar HWDGE
    queue, w1t-v on gpsimd; x_hd (residual copy) after.

Pitfalls baked into this design (cost a lot of debugging): custom-DVE
ops (reciprocal_approx_fast) and ISA ops (partition_broadcast) have
access patterns that are INVISIBLE to the Tile dependency tracker, and
post-hoc add_dep_helper edges do not take - they race and corrupt
results. TensorHandle.bitcast views ARE tracked. Matmul psum output
must be fp32. Only sync/scalar (HWDGE) and gpsimd (SWDGE, slow) can
issue DMAs; SWDGE transfers run ~3-6x slower.

Matmul inputs are bf16 (fp32 PSUM accumulate); w1 transposed and cast
on the host (pure layout prep).
"""
import numpy as np

GROUPS = 32
HEADS = 8
EPS = 1e-5
C = 512
T = 1024
CH = C // HEADS            # 64
SCALE = float(CH) ** -0.25
N_CORES = 8

DVE_EXP_UNITS = {(2, 0), (6, 1)}   # (st, n) units whose exp runs on DVE
EPI_V2 = True              # bisect: v2-style epilogue (DMA r-chain)
LAST_INLINE = True        # bisect: v2-style deferred last pair
N_WARM = 12                # PE warmup matmuls (HAM un-throttle)

LOG2E_128 = 184.6650558    # log2(e) * 128  (bf16 exponent scale)
SCHRAUD_B = 16250.3        # 127*128 - 5.7 (centres the 2^frac error band)


def _build_nc():
    import concourse.bass as bass
    import concourse.mybir as mybir
    import concourse.tile as tile
    from concourse import bacc
    from concourse.tile_rust import add_dep_helper

    f32 = mybir.dt.float32
    bf16 = mybir.dt.bfloat16
    i16 = mybir.dt.int16
    i32 = mybir.dt.int32
    Alu = mybir.AluOpType
    Act = mybir.ActivationFunctionType

    nc = bacc.Bacc("TRN2", target_bir_lowering=False, debug=False)

    x_d = nc.declare_dram_parameter("x", [C, T], f32, isOutput=False)
    w1t_d = nc.declare_dram_parameter("w1t", [C, 3 * C], bf16, isOutput=False)
    b1r_d = nc.declare_dram_parameter("b1r", [128, 8], f32, isOutput=False)
    b1vh_d = nc.declare_dram_parameter("b1vh", [64, 8], f32, isOutput=False)
    gnw_d = nc.declare_dram_parameter("gnw", [128, 4], f32, isOutput=False)
    gnb_d = nc.declare_dram_parameter("gnb", [128, 4], f32, isOutput=False)
    ind16_d = nc.declare_dram_parameter("ind16", [128, 8], f32, isOutput=False)
    indT_d = nc.declare_dram_parameter("indT", [8, 128], f32, isOutput=False)
    out_d = nc.declare_dram_parameter("out", [C, T], f32, isOutput=True)

    with tile.TileContext(nc) as tc:
        with (
            tc.tile_pool(name="cst", bufs=1) as cst,
            tc.tile_pool(name="work", bufs=2) as work,
            tc.tile_pool(name="wtp", bufs=4) as wtp,
            tc.tile_pool(name="outp", bufs=4) as outp,
            tc.tile_pool(name="ps", bufs=1, space="PSUM") as ps,
        ):
            # ---------------- PE warmup (no data deps) ----------------
            wuA = cst.tile([128, 128], bf16)
            nc.vector.memset(wuA, 0.0)
            wuB = cst.tile([128, 512], bf16)
            nc.vector.memset(wuB, 0.0)
            wu_ps = ps.tile([64, 512], f32, tag="av", bufs=3, name="warm")
            for i in range(N_WARM):
                nc.tensor.matmul(
                    out=wu_ps[0:64, :], lhsT=wuA[:, 0:64], rhs=wuB,
                    start=True, stop=True
                )

            # ---------------- loads ----------------
            # tiny consts first on the gpsimd queue
            b1r_sb = cst.tile([128, 8], f32)
            nc.gpsimd.dma_start(out=b1r_sb, in_=b1r_d[:, :])
            b1vh_sb = cst.tile([64, 8], f32)
            nc.gpsimd.dma_start(out=b1vh_sb, in_=b1vh_d[:, :])
            gnw_sb = cst.tile([128, 4], f32)
            nc.gpsimd.dma_start(out=gnw_sb, in_=gnw_d[:, :])
            gnb_sb = cst.tile([128, 4], f32)
            nc.gpsimd.dma_start(out=gnb_sb, in_=gnb_d[:, :])
            ind16 = cst.tile([128, 8], f32)
            nc.gpsimd.dma_start(out=ind16, in_=ind16_d[:, :])
            indT = cst.tile([8, 128], f32)
            nc.gpsimd.dma_start(out=indT, in_=indT_d[:, :])

            # x on sync+gpsimd; w1t q/k on the scalar queue (boot critical path)
            xv = x_d.ap().rearrange("(i p) t -> i p t", p=128)
            x_sb = cst.tile([128, 4, T], f32)
            w1t_sb = cst.tile([128, 4, 3 * C], bf16)
            w1tv = w1t_d.ap().rearrange("(i p) o -> p i o", p=128)
            nc.sync.dma_start(out=x_sb[:, 0, :], in_=xv[0])
            nc.sync.dma_start(out=x_sb[:, 1, :], in_=xv[1])
            nc.scalar.dma_start(out=x_sb[:, 2, :], in_=xv[2])
            nc.scalar.dma_start(out=x_sb[:, 3, :], in_=xv[3])
            nc.sync.dma_start(out=w1t_sb[:, :, 0:C], in_=w1tv[:, :, 0:C])
            nc.scalar.dma_start(out=w1t_sb[:, :, C : 2 * C], in_=w1tv[:, :, C : 2 * C])
            nc.gpsimd.dma_start(out=w1t_sb[:, :, 2 * C :], in_=w1tv[:, :, 2 * C :])
            # head-aligned residual copy of x (needed ~25us in)
            x_hd = cst.tile([64, 8, T], f32)
            nc.sync.dma_start(out=x_hd, in_=x_d.ap().rearrange("(h p) t -> p h t", p=64))

            # residual with v-bias prefolded: x_hd2[:,h,:] = x_hd[:,h,:] + b1v[h]
            x_hd2 = cst.tile([64, 8, T], f32)
            for hh in range(8):
                nc.vector.tensor_scalar(
                    out=x_hd2[:, hh, :], in0=x_hd[:, hh, :],
                    scalar1=b1vh_sb[:, hh : hh + 1], scalar2=None, op0=Alu.add,
                )

            # preload the exp table while DMAs stream
            dumm = cst.tile([8, 2], f32)
            nc.vector.memset(dumm, 0.0)
            nc.scalar.activation(
                out=dumm[:, 1:2], in_=dumm[:, 0:1], func=Act.Exp, bias=0.0, scale=1.0
            )

            # ---------------- GroupNorm ----------------
            rhs3 = cst.tile([128, 4, 3], f32)
            for i in range(4):
                st6 = work.tile([128, 2, 6], f32, tag="st6")
                nc.vector.bn_stats(out=st6[:, 0, :], in_=x_sb[:, i, 0:512])
                nc.vector.bn_stats(out=st6[:, 1, :], in_=x_sb[:, i, 512:1024])
                mv = work.tile([128, 2], f32, tag="mv")
                nc.vector.bn_aggr(out=mv, in_=st6)
                nc.vector.tensor_copy(out=rhs3[:, i, 0:2], in_=mv)
                nc.vector.tensor_mul(rhs3[:, i, 2:3], mv[:, 0:1], mv[:, 0:1])

            stats_ps = ps.tile([8, 12], f32, tag="qkv", name="stats_ps")
            for i in range(4):
                nc.tensor.matmul(
                    out=stats_ps[:, 3 * i : 3 * i + 3],
                    lhsT=ind16,
                    rhs=rhs3[:, i, :],
                    start=True,
                    stop=True,
                )
            sg = cst.tile([8, 12], f32)
            nc.vector.tensor_copy(out=sg, in_=stats_ps)
            musig = cst.tile([8, 2, 4], f32)
            mu_v = sg.rearrange("p (i three) -> p i three", three=3)
            nc.vector.tensor_copy(out=musig[:, 0, :], in_=mu_v[:, :, 0])
            var_g = cst.tile([8, 4], f32)
            nc.vector.tensor_add(var_g, mu_v[:, :, 1], mu_v[:, :, 2])
            mu2 = cst.tile([8, 4], f32)
            nc.vector.tensor_mul(mu2, mu_v[:, :, 0], mu_v[:, :, 0])
            nc.vector.tensor_sub(var_g, var_g, mu2)
            # rstd = 1/sqrt(var+eps): bit-trick seed + 2 Newton steps (DVE)
            ve = cst.tile([8, 4], f32)
            nc.vector.tensor_scalar(
                out=ve, in0=var_g, scalar1=EPS, scalar2=None, op0=Alu.add
            )
            t_i = cst.tile([8, 4], i32)
            nc.vector.tensor_scalar(
                out=t_i, in0=ve.bitcast(i32), scalar1=1, scalar2=None,
                op0=Alu.logical_shift_right,
            )
            t_x = cst.tile([8, 4], i32)
            nc.vector.tensor_scalar(
                out=t_x, in0=t_i, scalar1=-1, scalar2=None, op0=Alu.bitwise_xor
            )
            y_i = cst.tile([8, 4], i32)
            nc.vector.tensor_scalar(
                out=y_i, in0=t_x, scalar1=0x5F3759E0, scalar2=None, op0=Alu.add
            )
            y0 = y_i.bitcast(f32)
            t2 = cst.tile([8, 4], f32)
            nc.vector.tensor_mul(t2, y0, y0)
            nc.vector.tensor_mul(t2, t2, ve)
            nc.vector.tensor_scalar(
                out=t2, in0=t2, scalar1=-0.5, scalar2=1.5, op0=Alu.mult, op1=Alu.add
            )
            y1 = cst.tile([8, 4], f32)
            nc.vector.tensor_mul(y1, y0, t2)
            t3 = cst.tile([8, 4], f32)
            nc.vector.tensor_mul(t3, y1, y1)
            nc.vector.tensor_mul(t3, t3, ve)
            nc.vector.tensor_scalar(
                out=t3, in0=t3, scalar1=-0.5, scalar2=1.5, op0=Alu.mult, op1=Alu.add
            )
            nc.vector.tensor_mul(musig[:, 1, :], y1, t3)

            # broadcast (mu, rstd) to channels; fold gn affine; xn bf16
            xn_sb = cst.tile([128, 4, T], bf16)
            af = cst.tile([128, 4, 2], f32)
            for i in range(4):
                musig_ps = ps.tile([128, 2], f32, tag="av", bufs=3, name=f"musig_ps{i}")
                nc.tensor.matmul(
                    out=musig_ps, lhsT=indT, rhs=musig[:, :, i], start=True, stop=True
                )
                nc.vector.tensor_mul(af[:, i, 0:1], gnw_sb[:, i : i + 1], musig_ps[:, 1:2])
                tmp = work.tile([128, 1], f32, tag="tmp1")
                nc.vector.tensor_mul(tmp, musig_ps[:, 0:1], af[:, i, 0:1])
                nc.vector.tensor_sub(af[:, i, 1:2], gnb_sb[:, i : i + 1], tmp)
                nc.vector.tensor_scalar(
                    out=xn_sb[:, i, :],
                    in0=x_sb[:, i, :],
                    scalar1=af[:, i, 0:1],
                    scalar2=af[:, i, 1:2],
                    op0=Alu.mult,
                    op1=Alu.add,
                )

            # ---------------- QKV machinery ----------------
            q_sb = cst.tile([128, 4, T], bf16)
            k_sb = cst.tile([128, 4, T], bf16)
            vt_sb = cst.tile([128, 8, 8, 65], bf16)
            nc.vector.memset(vt_sb[:, :, :, 64:65], 1.0)

            def emit_qk_boot(j):
                # prologue q_j/k_j through the sc slots in n-halves: the n=0
                # halves land first, unblocking scores for st 0-3 early
                for n in range(2):
                    for oc, dst in ((j, q_sb), (4 + j, k_sb)):
                        qp = ps.tile(
                            [128, 512], f32, tag="sc", bufs=2, name=f"boot_{oc}_{n}"
                        )
                        for i in range(4):
                            nc.tensor.matmul(
                                out=qp,
                                lhsT=w1t_sb[:, i, 128 * oc : 128 * oc + 128],
                                rhs=xn_sb[:, i, 512 * n : 512 * n + 512],
                                start=(i == 0),
                                stop=(i == 3),
                            )
                        nc.vector.tensor_scalar(
                            out=dst[:, j, 512 * n : 512 * n + 512],
                            in0=qp,
                            scalar1=SCALE,
                            scalar2=b1r_sb[:, oc : oc + 1],
                            op0=Alu.mult,
                            op1=Alu.add,
                        )

            def emit_qk(j):
                # steady-state q_j/k_j in [128,512] halves through the staging slot
                for oc, dst in ((j, q_sb), (4 + j, k_sb)):
                    for n in range(2):
                        qp = ps.tile([128, 512], f32, tag="qkv", name=f"qk_{oc}_{n}")
                        for i in range(4):
                            nc.tensor.matmul(
                                out=qp,
                                lhsT=w1t_sb[:, i, 128 * oc : 128 * oc + 128],
                                rhs=xn_sb[:, i, 512 * n : 512 * n + 512],
                                start=(i == 0),
                                stop=(i == 3),
                            )
                        nc.vector.tensor_scalar(
                            out=dst[:, j, 512 * n : 512 * n + 512],
                            in0=qp,
                            scalar1=SCALE,
                            scalar2=b1r_sb[:, oc : oc + 1],
                            op0=Alu.mult,
                            op1=Alu.add,
                        )

            def emit_v(st):
                vp = ps.tile([128, 512], f32, tag="av", bufs=3, name=f"v_{st}")
                for i in range(4):
                    nc.tensor.matmul(
                        out=vp,
                        lhsT=xn_sb[:, i, 128 * st : 128 * st + 128],
                        rhs=w1t_sb[:, i, 2 * C : 3 * C],
                        start=(i == 0),
                        stop=(i == 3),
                    )
                nc.vector.tensor_copy(
                    out=vt_sb[:, st, :, 0:64],
                    in_=vp.rearrange("p (h c) -> p h c", c=64),
                )

            wt_exp_inst = {}

            ones64 = cst.tile([1, 64], f32)
            nc.vector.memset(ones64, 1.0)

            def epilogue(j, h, n, avt, mm_last):
                # evacuate av+r at once (frees the av psum slot immediately);
                # 1/r via [16,32] reshape; broadcast 1/r to 64 partitions with
                # a K=1 PE matmul into psum (no slow DMA broadcast).
                o65 = outp.tile([65, 512], bf16, tag="o65", bufs=4, name=f"o65_{h}_{n}")
                nc.vector.tensor_copy(out=o65, in_=avt[0:65, :])
                rsp = wtp.tile([16, 32], bf16, tag="rsp", bufs=4, name=f"rp_{h}_{n}")
                nc.sync.dma_start(out=rsp, in_=o65[64:65, :])
                rsp2 = wtp.tile([16, 32], f32, tag="rsp2", bufs=4, name=f"rq_{h}_{n}")
                nc.vector.reciprocal(out=rsp2, in_=rsp)
                rrow2 = wtp.tile([1, 512], f32, tag="rrow2", bufs=4, name=f"r2_{h}_{n}")
                nc.sync.dma_start(out=rrow2, in_=rsp2)
                rbc_ps = ps.tile([64, 512], f32, tag="av", bufs=3, name=f"rb_{h}_{n}")
                nc.tensor.matmul(
                    out=rbc_ps, lhsT=ones64, rhs=rrow2, start=True, stop=True
                )
                o_f = outp.tile([64, 512], f32, tag="obf", name=f"ob_{h}_{n}")
                nc.vector.tensor_mul(o_f, o65[0:64, :], rbc_ps)
                out_f = outp.tile([64, 512], f32, tag="of", name=f"of_{h}_{n}")
                if j == HEADS // 2 - 1:
                    # tail is latency-bound: DVE add is 2x faster than gpsimd
                    nc.vector.tensor_add(
                        out_f, o_f, x_hd2[:, h, 512 * n : 512 * n + 512]
                    )
                else:
                    nc.gpsimd.tensor_add(
                        out_f, o_f, x_hd2[:, h, 512 * n : 512 * n + 512]
                    )
                nc.sync.dma_start(
                    out=out_d[64 * h : 64 * h + 64, 512 * n : 512 * n + 512], in_=out_f
                )

            # ---------------- pipeline ----------------
            emit_qk_boot(0)

            for j in range(HEADS // 2):
                hA, hB = 2 * j, 2 * j + 1
                last = (j == HEADS // 2 - 1) and LAST_INLINE
                av = {
                    (hA, 0): ps.tile([65, 512], f32, tag="av", bufs=3, name=f"av_{hA}_0"),
                    (hB, 0): ps.tile([65, 512], f32, tag="av", bufs=3, name=f"av_{hB}_0"),
                }
                av_last = {}
                if last:
                    # borrow the idle staging slot for a 4th inline accumulator
                    av[(hA, 1)] = ps.tile([128, 512], f32, tag="qkv", name=f"av_{hA}_1")
                    av[(hB, 1)] = ps.tile([65, 512], f32, tag="av", bufs=3, name=f"av_{hB}_1")
                wts = []

                def emit_av(st, n_range):
                    for n in n_range:
                        for hi, h in enumerate((hA, hB)):
                            mm = nc.tensor.matmul(
                                out=av[(h, n)][0:65, :],
                                lhsT=vt_sb[:, st, h, 0:65],
                                rhs=wts[st][n][:, 512 * hi : 512 * hi + 512],
                                start=(st == 0),
                                stop=(st == 7),
                            )
                            if st == 7:
                                av_last[(h, n)] = mm

                for st in range(8):
                    if j == 0:
                        emit_v(st)
                    wt_pair = []
                    for n in range(2):
                        scn = ps.tile([128, T], f32, tag="sc", bufs=2, name=f"sc_{j}_{st}_{n}")
                        for hi, h in enumerate((hA, hB)):
                            hp = 64 * hi
                            nc.tensor.matmul(
                                out=scn[:, 512 * hi : 512 * hi + 512],
                                lhsT=k_sb[hp : hp + 64, j, 128 * st : 128 * st + 128],
                                rhs=q_sb[hp : hp + 64, j, 512 * n : 512 * n + 512],
                                start=True,
                                stop=True,
                                tile_position=(hp, 0),
                            )
                        wtn = wtp.tile(
                            [128, T], bf16, tag="wt", bufs=20, name=f"wt_{j}_{st}_{n}"
                        )
                        if (st, n) in DVE_EXP_UNITS:
                            # write through a bitcast handle: untracked by Tile,
                            # so AV reads get explicit deps (wt_exp_inst)
                            ei = nc.vector.tensor_scalar(
                                out=wtn.bitcast(i16),
                                in0=scn,
                                scalar1=LOG2E_128,
                                scalar2=SCHRAUD_B,
                                op0=Alu.mult,
                                op1=Alu.add,
                            )
                            wt_exp_inst[id(wtn)] = ei
                        else:
                            nc.scalar.activation(
                                out=wtn, in_=scn, func=Act.Exp, bias=0.0, scale=1.0
                            )
                        wt_pair.append(wtn)
                    wts.append(wt_pair)

                    # AV deferred by one st so the PE FIFO never waits on the
                    # exp of the st it just scored (head-of-line blocking)
                    n_range = (0, 1) if last else (0,)
                    if st > 0:
                        emit_av(st - 1, n_range)
                if True:
                    emit_av(7, n_range)

                if not last:
                    epilogue(j, hA, 0, av[(hA, 0)], av_last[(hA, 0)])
                    av[(hA, 1)] = ps.tile([65, 512], f32, tag="av", bufs=3, name=f"av_{hA}_1")
                    epilogue(j, hB, 0, av[(hB, 0)], av_last[(hB, 0)])
                    av[(hB, 1)] = ps.tile([65, 512], f32, tag="av", bufs=3, name=f"av_{hB}_1")
                    if j < HEADS // 2 - 1:
                        emit_qk(j + 1)
                    for st in range(8):
                        for hi, h in enumerate((hA, hB)):
                            mm = nc.tensor.matmul(
                                out=av[(h, 1)][0:65, :],
                                lhsT=vt_sb[:, st, h, 0:65],
                                rhs=wts[st][1][:, 512 * hi : 512 * hi + 512],
                                start=(st == 0),
                                stop=(st == 7),
                            )
                            if st == 7:
                                av_last[(h, 1)] = mm
                    epilogue(j, hA, 1, av[(hA, 1)], av_last[(hA, 1)])
                    epilogue(j, hB, 1, av[(hB, 1)], av_last[(hB, 1)])
                else:
                    for h in (hA, hB):
                        for n in (0, 1):
                            epilogue(j, h, n, av[(h, n)], av_last[(h, n)])

    nc.finalize()
    return nc


def _make_in_maps(inputs):
    x = np.ascontiguousarray(np.asarray(inputs["x"], dtype=np.float32))
    gnw = np.asarray(inputs["gn_weight"], dtype=np.float32)
    gnb = np.asarray(inputs["gn_bias"], dtype=np.float32)
    w1 = np.asarray(inputs["w1"], dtype=np.float32)
    b1 = np.asarray(inputs["b1"], dtype=np.float32)

    import ml_dtypes

    B = x.shape[0]
    w1t = np.ascontiguousarray(w1[:, :, 0].T).astype(ml_dtypes.bfloat16)  # [C, 3C]
    b1r = np.ascontiguousarray(b1[: 2 * C].reshape(8, 128).T) * (float(CH) ** -0.25)  # [128, 8], pre-scaled
    b1vh = np.ascontiguousarray(b1[2 * C :].reshape(8, 64).T)       # [64, 8]
    gnw_r = np.ascontiguousarray(gnw.reshape(4, 128).T)             # [128, 4]
    gnb_r = np.ascontiguousarray(gnb.reshape(4, 128).T)             # [128, 4]

    ind16 = np.zeros((128, 8), np.float32)
    indT = np.zeros((8, 128), np.float32)
    for g in range(8):
        ind16[16 * g : 16 * g + 16, g] = 1.0 / 16.0
        indT[g, 16 * g : 16 * g + 16] = 1.0

    in_maps = []
    for b in range(B):
        in_maps.append(
            {
                "x": np.ascontiguousarray(x[b].reshape(C, T)),
                "w1t": w1t,
                "b1r": b1r,
                "b1vh": b1vh,
                "gnw": gnw_r,
                "gnb": gnb_r,
                "ind16": ind16,
                "indT": indT,
            }
        )
    return in_maps


def _gather(results, x_shape):
    B, Cc, H, W = x_shape
    out = np.empty((B, Cc, H, W), dtype=np.float32)
    for b in range(B):
        out[b] = results[b]["out"].reshape(Cc, H, W)
    return out


def kernel(**inputs):
    from concourse.bass_utils import run_bass_kernel_spmd

    nc = _build_nc()
    in_maps = _make_in_maps(inputs)
    res = run_bass_kernel_spmd(nc, in_maps, core_ids=list(range(N_CORES)))
    return _gather(res.results, np.asarray(inputs["x"]).shape)
